# revision 10
# baseline (speedup 1.0000x reference)
"""Additive attention kernel for 8 Trainium2 NeuronCores.

Math: scores[b,i,j] = sum_d tanh(q[b,i,d] + k[b,j,d]); out = softmax_j(scores) @ v.

Key trick: tanh(s) ~= sum_m C[m] sin(W[m] s) (M=8 fit on |s|<=9.8; end-to-end rel
err ~1.5e-3), and sin(w(q+k)) = sin(wq)cos(wk) + cos(wq)sin(wk) is separable, so
the [512 x 512 x 64]-point tanh reduction becomes a rank-1024 PE matmul:
  scoresT[jb] = sum_chunks FK_chunk^T @ FQ_chunk.
Softmax needs no max-subtraction (scores in [-25, 24]); the denominator comes
from a ones-column appended to V in the AV matmul; DVE reciprocal normalizes.

Angle range reduction (ScalarE Sin is only valid on ~[-pi, pi], args reach 22):
  pre = x*(w/2pi) + 0.25*is_cos      (DVE; cos shifts the rounding point)
  n   = (pre + MAGIC) - MAGIC        (DVE; f32 adds round to nearest integer)
  red = diag(w) @ x - 2pi*I @ n      (PE, accumulated in PSUM)
  feat= Sin(red [+ pi/2 bias])       (ScalarE, PSUM -> SBUF)
The K side swaps sin/cos so chunk products give sin(wq)cos(wk) + cos(wq)sin(wk).
Amplitudes fold in as Identity acts with per-partition scale (same table set).

Sharding: B=8 -> one batch per core, no collectives.
"""

import math

import numpy as np

import concourse.bass as bass
import concourse.mybir as mybir
from concourse.bass_utils import run_bass_kernel_spmd

F32 = mybir.dt.float32
AF = mybir.ActivationFunctionType
ALU = mybir.AluOpType

# tanh(s) ~= sum_m C[m] * sin(W[m] * s), fit on s in [-9.8, 9.8], N(0, sqrt2) weight
W = [0.273822509, 0.825679394, 1.38832881, 1.96485759,
     2.55624192, 3.16272728, 3.77941797, 4.47596827]
C = [1.23648196, 0.32878853, 0.13027896, 0.0534403941,
     0.0215942849, 0.00858971558, 0.00318991782, 0.00161968402]

B, L, D = 8, 512, 64
PI = math.pi
TWO_PI = 2.0 * math.pi
MAGIC = 12582912.0  # 1.5 * 2^23: adding rounds f32 to nearest integer

PAIRS = [(0, 1), (2, 3), (4, 5), (6, 7)]  # freqs packed on partition halves

_CACHE = {}


def _build():
    nc = bass.Bass()
    qt2_ext = nc.declare_dram_parameter("qt2", [128, L], F32, isOutput=False)
    kt2_ext = nc.declare_dram_parameter("kt2", [128, L], F32, isOutput=False)
    vh_ext = nc.declare_dram_parameter("vh", [L, 65], F32, isOutput=False)
    tbl_ext = nc.declare_dram_parameter("tbl", [128, 9], F32, isOutput=False)
    diags_ext = nc.declare_dram_parameter("diags", [128, 5, 128], F32, isOutput=False)
    out_ext = nc.declare_dram_parameter("out", [L, D], F32, isOutput=True)

    from contextlib import ExitStack

    with ExitStack() as ctx:
        e = ctx.enter_context
        QT2 = e(nc.sbuf_tensor([128, L], F32))
        KT2 = e(nc.sbuf_tensor([128, L], F32))
        TBL = e(nc.sbuf_tensor([128, 9], F32))
        DIAGS = e(nc.sbuf_tensor([128, 5, 128], F32))
        VH = e(nc.sbuf_tensor([128, 4, 65], F32))
        # magic-round intermediates [side, pair, trig]: pre and n
        PRE = e(nc.sbuf_tensor([128, 2, 4, 2, L], F32))
        NN = e(nc.sbuf_tensor([128, 2, 4, 2, L], F32))
        FQRAW = e(nc.sbuf_tensor([128, 4, 2, L], F32))
        FQS = e(nc.sbuf_tensor([128, 4, 2, L], F32))
        FK = e(nc.sbuf_tensor([128, 4, 2, L], F32))
        EXPT = e(nc.sbuf_tensor([128, 4, L], F32))
        RCP = e(nc.sbuf_tensor([128, 4], F32))
        OUT = e(nc.sbuf_tensor([128, 4, D], F32))
        PSUMS = e(nc.psum_tensor([128, 4 * L], F32))   # banks for scoresT
        PSUMR = e(nc.psum_tensor([128, 4, L], F32))    # red rotation; later AV out
        s_in = e(nc.semaphore("s_in"))
        s_vh = e(nc.semaphore("s_vh"))
        s_n2 = e(nc.semaphore("s_n2"))
        s_red = e(nc.semaphore("s_red"))
        s_sin = e(nc.semaphore("s_sin"))
        s_amp = e(nc.semaphore("s_amp"))
        s_scores = e(nc.semaphore("s_scores"))
        s_exp = e(nc.semaphore("s_exp"))
        s_av = e(nc.semaphore("s_av"))
        s_norm = e(nc.semaphore("s_norm"))
        block = e(nc.Block())

        # red order: for pair j: Q t=0 (sin), Q t=1 (cos), K t=0 (cos), K t=1 (sin)
        RED_SEQ = [(j, s, t) for j in range(4) for s in range(2) for t in range(2)]

        def is_cos(s, t):
            return t == 1 if s == 0 else t == 0

        @block.sync
        def _(sync):
            sync.dma_start(out=TBL[:], in_=tbl_ext[:]).then_inc(s_in, 16)
            sync.dma_start(out=DIAGS[:], in_=diags_ext[:]).then_inc(s_in, 16)
            sync.dma_start(out=QT2[:], in_=qt2_ext[:]).then_inc(s_in, 16)
            sync.dma_start(out=KT2[:], in_=kt2_ext[:]).then_inc(s_in, 16)
            sync.dma_start(
                out=VH[:], in_=vh_ext.rearrange("(g p) c -> p g c", p=128)
            ).then_inc(s_vh, 16)
            sync.wait_ge(s_norm, 4)
            sync.dma_start(
                out=out_ext.rearrange("(g p) c -> p g c", p=128), in_=OUT[:]
            ).then_inc(s_in, 16)

        @block.vector
        def _(vector):
            vector.wait_ge(s_in, 64)
            for j, s, t in RED_SEQ:
                X2 = QT2 if s == 0 else KT2
                shift = 0.25 if is_cos(s, t) else 0.0
                vector.tensor_scalar(
                    PRE[:, s, j, t, :], X2[:], TBL[:, j : j + 1], shift,
                    ALU.mult, ALU.add,
                )
                vector.tensor_scalar(
                    NN[:, s, j, t, :], PRE[:, s, j, t, :], MAGIC, -MAGIC,
                    ALU.add, ALU.add,
                ).then_inc(s_n2, 1)
            for ib in range(4):
                vector.wait_ge(s_av, ib + 1)
                vector.reciprocal(RCP[:, ib : ib + 1], PSUMR[:, ib, 64:65])
            vector.drain()
            for ib in range(4):
                vector.tensor_scalar_mul(
                    OUT[:, ib, :], PSUMR[:, ib, 0:D], RCP[:, ib : ib + 1]
                ).then_inc(s_norm, 1)

        @block.scalar
        def _(scalar):
            # sins follow red production order; amp after each pair's Q sins
            for g, (j, s, t) in enumerate(RED_SEQ):
                scalar.wait_ge(s_red, g + 1)
                dst = FQRAW[:, j, t, :] if s == 0 else FK[:, j, t, :]
                if is_cos(s, t):
                    scalar.activation(
                        dst, PSUMR[:, g % 4, :], AF.Sin, bias=TBL[:, 8:9]
                    ).then_inc(s_sin, 1)
                else:
                    scalar.activation(
                        dst, PSUMR[:, g % 4, :], AF.Sin
                    ).then_inc(s_sin, 1)
                if s == 0 and t == 1:
                    scalar.activation(
                        FQS[:, j], FQRAW[:, j], AF.Identity,
                        scale=TBL[:, 4 + j : 5 + j],
                    ).then_inc(s_amp, 1)
            scalar.wait_ge(s_scores, 1)
            scalar.activation(EXPT[:], PSUMS[:], AF.Exp).then_inc(s_exp, 1)

        @block.tensor
        def _(tensor):
            # reds (2 mms each, rotating PSUMR banks) interleaved with scores
            def red(g):
                j, s, t = RED_SEQ[g]
                X2 = QT2 if s == 0 else KT2
                tensor.wait_ge(s_n2, g + 1)
                if g >= 4:
                    tensor.wait_ge(s_sin, g - 3)   # bank g%4 free again
                tensor.matmul(
                    PSUMR[:, g % 4, :], DIAGS[:, j, :], X2[:],
                    start=True, stop=False,
                )
                tensor.matmul(
                    PSUMR[:, g % 4, :], DIAGS[:, 4, :], NN[:, s, j, t, :],
                    start=False, stop=True,
                ).then_inc(s_red, 1)

            def scores(j):
                tensor.wait_ge(s_amp, j + 1)
                tensor.wait_ge(s_sin, 4 * j + 4)
                for t in range(2):
                    for jb in range(4):
                        mm = tensor.matmul(
                            PSUMS[:, jb * L : (jb + 1) * L],
                            FK[:, j, t, jb * 128 : (jb + 1) * 128],
                            FQS[:, j, t, :],
                            start=(j == 0 and t == 0),
                            stop=(j == 3 and t == 1),
                        )
                return mm

            for g in range(4):
                red(g)
            for j in range(4):
                for g in range(4 * (j + 1), 4 * (j + 2)):
                    if g < 16:
                        red(g)
                mm = scores(j)
            mm.then_inc(s_scores, 1)

            tensor.wait_ge(s_exp, 1)
            tensor.wait_ge(s_vh, 16)
            for ib in range(4):
                for jb in range(4):
                    mm = tensor.matmul(
                        PSUMR[:, ib, 0:65],
                        EXPT[:, jb, ib * 128 : (ib + 1) * 128],
                        VH[:, jb, :],
                        start=(jb == 0),
                        stop=(jb == 3),
                    )
                mm.then_inc(s_av, 1)

    return nc


def _get_nc():
    if "nc" not in _CACHE:
        _CACHE["nc"] = _build()
    return _CACHE["nc"]


def _make_consts():
    tbl = np.zeros((128, 9), np.float32)
    tbl[:, 8] = np.float32(math.pi / 2)
    diags = np.zeros((128, 5, 128), np.float32)
    for j, (a, b) in enumerate(PAIRS):
        tbl[0:64, j] = W[a] / TWO_PI
        tbl[64:128, j] = W[b] / TWO_PI
        tbl[0:64, 4 + j] = C[a]
        tbl[64:128, 4 + j] = C[b]
        for p in range(64):
            diags[p, j, p] = W[a]
            diags[64 + p, j, 64 + p] = W[b]
    for p in range(128):
        diags[p, 4, p] = -TWO_PI
    return tbl, diags


def _make_in_maps(q, k, v):
    tbl, diags = _make_consts()
    in_maps = []
    for b in range(B):
        qt = np.ascontiguousarray(q[b].T.astype(np.float32))   # [64, 512]
        kt = np.ascontiguousarray(k[b].T.astype(np.float32))
        qt2 = np.concatenate([qt, qt], axis=0)                  # [128, 512]
        kt2 = np.concatenate([kt, kt], axis=0)
        vh = np.concatenate(
            [v[b].astype(np.float32), np.ones((L, 1), np.float32)], axis=1
        )
        in_maps.append(
            {"qt2": qt2, "kt2": kt2, "vh": vh, "tbl": tbl, "diags": diags}
        )
    return in_maps


def _run(in_maps, **kw):
    nc = _get_nc()
    return run_bass_kernel_spmd(nc, in_maps, core_ids=list(range(8)), **kw)


def kernel(q: np.ndarray, k: np.ndarray, v: np.ndarray) -> np.ndarray:
    res = _run(_make_in_maps(q, k, v))
    out = np.stack([res.results[b]["out"] for b in range(B)]).astype(np.float32)
    return out


# revision 12
# speedup vs baseline: 1.6058x; 1.6058x over previous
"""Additive attention kernel for 8 Trainium2 NeuronCores.

Math: scores[b,i,j] = sum_d tanh(q[b,i,d] + k[b,j,d]); out = softmax_j(scores) @ v.

tanh(s) ~= sum_m C[m] sin(W[m] s) (M=8, refit with bf16-exact W[m]/2pi), and
sin(w(q+k)) = sin(wq)cos(wk) + cos(wq)sin(wk) is separable -> scores become a
rank-1024 PE matmul in bf16 (f32 matmul runs as two slow LOW_HIGH passes; bf16
is a single full-rate pass).

Angle path, in turns (t = w x / 2pi), all matmuls bf16 with exact operands:
  t0_psum   = diag(w/2pi) @ (x_hi + x_lo)      (PE; host splits x = hi+lo bf16)
  cos bank += 0.25 (rank-1 ones pass)          (PE; shifts the rounding point)
  n         = (t0 + MAGIC) - MAGIC             (DVE, f32 magic round -> bf16 ints)
  red_psum += (-I) @ n                         (PE; red in [-0.5, 0.5] turns)
  feat      = Sin(2pi * red)  -> bf16          (ScalarE, PSUM -> SBUF, pair-merged)
K-side banks are swapped (cos first) so chunk products pair sin with cos.
Amplitudes: DVE bf16 tensor_scalar with per-partition table. Softmax without
max-subtraction; denominator via a ones-column in V; DVE reciprocal normalizes.

Sharding: B=8 -> one batch per core, no collectives.
"""

import math

import numpy as np
import ml_dtypes

import concourse.bass as bass
import concourse.mybir as mybir
from concourse.bass_utils import run_bass_kernel_spmd

F32 = mybir.dt.float32
BF16 = mybir.dt.bfloat16
AF = mybir.ActivationFunctionType
ALU = mybir.AluOpType

# base fit (amplitudes refit below against bf16-exact frequencies)
W0 = [0.273822509, 0.825679394, 1.38832881, 1.96485759,
      2.55624192, 3.16272728, 3.77941797, 4.47596827]

B, L, D, M = 8, 512, 64, 8
PI = math.pi
TWO_PI = 2.0 * math.pi
MAGIC = 12582912.0  # 1.5 * 2^23

PAIRS = [(0, 1), (2, 3), (4, 5), (6, 7)]


def _bf(x):
    return np.asarray(x).astype(ml_dtypes.bfloat16)


def _fit_consts():
    w2pi = _bf(np.array(W0, np.float32) / TWO_PI).astype(np.float64)
    w_eff = w2pi * TWO_PI
    S = 9.8
    sg = np.linspace(-S, S, 4001)
    wts = np.exp(-(sg**2) / 4) + 0.02
    A = np.sin(np.outer(sg, w_eff)) * np.sqrt(wts)[:, None]
    c, *_ = np.linalg.lstsq(A, np.tanh(sg) * np.sqrt(wts), rcond=None)
    return w2pi.astype(np.float32), c.astype(np.float32)


W2PI, C = _fit_consts()

_CACHE = {}


def _build():
    nc = bass.Bass()
    qh_ext = nc.declare_dram_parameter("qh", [128, L], BF16, isOutput=False)
    ql_ext = nc.declare_dram_parameter("ql", [128, L], BF16, isOutput=False)
    kh_ext = nc.declare_dram_parameter("kh", [128, L], BF16, isOutput=False)
    kl_ext = nc.declare_dram_parameter("kl", [128, L], BF16, isOutput=False)
    vh_ext = nc.declare_dram_parameter("vh", [L, 65], BF16, isOutput=False)
    dg_ext = nc.declare_dram_parameter("dg", [128, 6, 128], BF16, isOutput=False)
    amp_ext = nc.declare_dram_parameter("amp", [128, 4], F32, isOutput=False)
    out_ext = nc.declare_dram_parameter("out", [L, D], F32, isOutput=True)

    from contextlib import ExitStack

    with ExitStack() as ctx:
        e = ctx.enter_context
        QH = e(nc.sbuf_tensor("QH", [128, L], BF16))
        QL = e(nc.sbuf_tensor("QL", [128, L], BF16))
        KH = e(nc.sbuf_tensor("KH", [128, L], BF16))
        KL = e(nc.sbuf_tensor("KL", [128, L], BF16))
        DG = e(nc.sbuf_tensor([128, 6, 128], BF16))  # 0-3: diag(w/2pi); 4: -I; 5: 0.25-col
        ONES = e(nc.sbuf_tensor([1, L], BF16))
        AMP = e(nc.sbuf_tensor([128, 4], F32))
        VH = e(nc.sbuf_tensor([128, 4, 65], BF16))
        NS = e(nc.sbuf_tensor([128, 2, 4, L], BF16))
        NC_ = e(nc.sbuf_tensor("NCT", [128, 2, 4, L], BF16))
        FQRAW = e(nc.sbuf_tensor([128, 4, 2, L], BF16))
        FQS = e(nc.sbuf_tensor([128, 4, 2, L], BF16))
        FK = e(nc.sbuf_tensor([128, 4, 2, L], BF16))
        EXPT = e(nc.sbuf_tensor([128, 4, L], BF16))
        RCP = e(nc.sbuf_tensor([128, 4], F32))
        OUT = e(nc.sbuf_tensor([128, 4, D], F32))
        PSUMS = e(nc.psum_tensor([128, 4 * L], F32))
        PSUMR = e(nc.psum_tensor([128, 4, L], F32))
        s_in = e(nc.semaphore("s_in"))
        s_vh = e(nc.semaphore("s_vh"))
        s_t0 = e(nc.semaphore("s_t0"))
        s_n = e(nc.semaphore("s_n"))
        s_red = e(nc.semaphore("s_red"))
        s_act = e(nc.semaphore("s_act"))
        s_amp = e(nc.semaphore("s_amp"))
        s_scores = e(nc.semaphore("s_scores"))
        s_exp = e(nc.semaphore("s_exp"))
        s_av = e(nc.semaphore("s_av"))
        s_norm = e(nc.semaphore("s_norm"))
        block = e(nc.Block())

        XH = [QH, KH]
        XL = [QL, KL]

        # units: g = 2*pair + side (Q first). Banks: rA=(2g)%4, rB=rA+1.
        # Q: A=sin, B=cos; K: A=cos, B=sin (so FK comes out [cos|sin]).
        def banks(g):
            rA = (2 * g) % 4
            return rA, rA + 1

        @block.sync
        def _(sync):
            sync.dma_start(out=DG[:], in_=dg_ext[:]).then_inc(s_in, 16)
            sync.dma_start(out=AMP[:], in_=amp_ext[:]).then_inc(s_in, 16)
            sync.dma_start(out=QH[:], in_=qh_ext[:]).then_inc(s_in, 16)
            sync.dma_start(out=QL[:], in_=ql_ext[:]).then_inc(s_in, 16)
            sync.dma_start(out=KH[:], in_=kh_ext[:]).then_inc(s_in, 16)
            sync.dma_start(out=KL[:], in_=kl_ext[:]).then_inc(s_in, 16)
            sync.dma_start(
                out=VH[:], in_=vh_ext.rearrange("(g p) c -> p g c", p=128)
            ).then_inc(s_vh, 16)
            sync.wait_ge(s_norm, 4)
            sync.dma_start(
                out=out_ext.rearrange("(g p) c -> p g c", p=128), in_=OUT[:]
            ).then_inc(s_in, 16)

        @block.gpsimd
        def _(gpsimd):
            gpsimd.memset(ONES[:], 1.0)
            gpsimd.sem_inc(s_in, 1)

        @block.vector
        def _(vector):
            for g in range(8):
                j, s = g // 2, g % 2
                rA, rB = banks(g)
                sin_bank, cos_bank = (rA, rB) if s == 0 else (rB, rA)
                vector.wait_ge(s_t0, 2 * g + 1)
                vector.tensor_scalar(
                    NS[:, s, j, :], PSUMR[:, sin_bank, :], MAGIC, -MAGIC,
                    ALU.add, ALU.add,
                ).then_inc(s_n, 1)
                vector.wait_ge(s_t0, 2 * g + 2)
                vector.tensor_scalar(
                    NC_[:, s, j, :], PSUMR[:, cos_bank, :], MAGIC, -MAGIC,
                    ALU.add, ALU.add,
                ).then_inc(s_n, 1)
                if s == 1:
                    vector.wait_ge(s_act, 2 * j + 1)
                    vector.tensor_scalar_mul(
                        FQS[:, j], FQRAW[:, j], AMP[:, j : j + 1]
                    ).then_inc(s_amp, 1)
            for ib in range(4):
                vector.wait_ge(s_av, ib + 1)
                vector.reciprocal(RCP[:, ib : ib + 1], PSUMR[:, ib, 64:65])
            vector.drain()
            for ib in range(4):
                vector.tensor_scalar_mul(
                    OUT[:, ib, :], PSUMR[:, ib, 0:D], RCP[:, ib : ib + 1]
                ).then_inc(s_norm, 1)

        @block.scalar
        def _(scalar):
            for g in range(8):
                j, s = g // 2, g % 2
                rA, _ = banks(g)
                dst = FQRAW[:, j] if s == 0 else FK[:, j]
                scalar.wait_ge(s_red, 2 * g + 2)
                scalar.activation(
                    dst, PSUMR[:, rA : rA + 2, :], AF.Sin, scale=TWO_PI
                ).then_inc(s_act, 1)
            scalar.wait_ge(s_scores, 1)
            scalar.activation(EXPT[:], PSUMS[:], AF.Exp).then_inc(s_exp, 1)

        @block.tensor
        def _(tensor):
            def red_unit(g):
                j, s = g // 2, g % 2
                rA, rB = banks(g)
                sin_bank, cos_bank = (rA, rB) if s == 0 else (rB, rA)
                tensor.wait_ge(s_in, 97)
                if g >= 2:
                    tensor.wait_ge(s_act, g - 1)  # banks free after act of g-2
                tensor.matmul(PSUMR[:, sin_bank, :], DG[:, j, :], XH[s][:],
                              start=True, stop=False)
                tensor.matmul(PSUMR[:, cos_bank, :], DG[:, j, :], XH[s][:],
                              start=True, stop=False)
                tensor.matmul(PSUMR[:, sin_bank, :], DG[:, j, :], XL[s][:],
                              start=False, stop=False).then_inc(s_t0, 1)
                tensor.matmul(PSUMR[:, cos_bank, :], DG[:, j, :], XL[s][:],
                              start=False, stop=False)
                tensor.matmul(PSUMR[:, cos_bank, :], DG[:1, 5, :], ONES[:],
                              start=False, stop=False).then_inc(s_t0, 1)
                tensor.wait_ge(s_n, 2 * g + 1)
                tensor.matmul(PSUMR[:, sin_bank, :], DG[:, 4, :], NS[:, s, j, :],
                              start=False, stop=True).then_inc(s_red, 1)
                tensor.wait_ge(s_n, 2 * g + 2)
                tensor.matmul(PSUMR[:, cos_bank, :], DG[:, 4, :], NC_[:, s, j, :],
                              start=False, stop=True).then_inc(s_red, 1)

            def scores(j):
                tensor.wait_ge(s_amp, j + 1)
                tensor.wait_ge(s_act, 2 * j + 2)
                for t in range(2):
                    for jb in range(4):
                        mm = tensor.matmul(
                            PSUMS[:, jb * L : (jb + 1) * L],
                            FK[:, j, t, jb * 128 : (jb + 1) * 128],
                            FQS[:, j, t, :],
                            start=(j == 0 and t == 0),
                            stop=(j == 3 and t == 1),
                        )
                return mm

            for g in range(2):
                red_unit(g)
            for j in range(4):
                for g in (2 * j + 2, 2 * j + 3):
                    if g < 8:
                        red_unit(g)
                mm = scores(j)
            mm.then_inc(s_scores, 1)

            tensor.wait_ge(s_exp, 1)
            tensor.wait_ge(s_vh, 16)
            for ib in range(4):
                for jb in range(4):
                    mm = tensor.matmul(
                        PSUMR[:, ib, 0:65],
                        EXPT[:, jb, ib * 128 : (ib + 1) * 128],
                        VH[:, jb, :],
                        start=(jb == 0),
                        stop=(jb == 3),
                    )
                mm.then_inc(s_av, 1)

    return nc


def _get_nc():
    if "nc" not in _CACHE:
        _CACHE["nc"] = _build()
    return _CACHE["nc"]


def _make_consts():
    dg = np.zeros((128, 6, 128), np.float32)
    amp = np.zeros((128, 4), np.float32)
    for j, (a, b) in enumerate(PAIRS):
        for p in range(64):
            dg[p, j, p] = W2PI[a]
            dg[64 + p, j, 64 + p] = W2PI[b]
        amp[0:64, j] = C[a]
        amp[64:128, j] = C[b]
    for p in range(128):
        dg[p, 4, p] = -1.0
    dg[0, 5, :] = 0.25
    return _bf(dg), amp


def _make_in_maps(q, k, v):
    dg, amp = _make_consts()
    in_maps = []
    for b in range(B):
        def hilo(x):
            xt = np.ascontiguousarray(x.T.astype(np.float32))      # [64, 512]
            x2 = np.concatenate([xt, xt], axis=0)                   # [128, 512]
            h = _bf(x2)
            lo = _bf(x2 - h.astype(np.float32))
            return h, lo

        qh, ql = hilo(q[b])
        kh, kl = hilo(k[b])
        vh = _bf(np.concatenate(
            [v[b].astype(np.float32), np.ones((L, 1), np.float32)], axis=1
        ))
        in_maps.append({"qh": qh, "ql": ql, "kh": kh, "kl": kl,
                        "vh": vh, "dg": dg, "amp": amp})
    return in_maps


def _run(in_maps, **kw):
    nc = _get_nc()
    return run_bass_kernel_spmd(nc, in_maps, core_ids=list(range(8)), **kw)


def kernel(q: np.ndarray, k: np.ndarray, v: np.ndarray) -> np.ndarray:
    res = _run(_make_in_maps(q, k, v))
    out = np.stack([res.results[b]["out"] for b in range(B)]).astype(np.float32)
    return out


# revision 13
# speedup vs baseline: 1.6946x; 1.0553x over previous
"""Additive attention kernel for 8 Trainium2 NeuronCores.

Math: scores[b,i,j] = sum_d tanh(q[b,i,d] + k[b,j,d]); out = softmax_j(scores) @ v.

tanh(s) ~= sum_m C[m] sin(W[m] s) (M=8, refit with bf16-exact W[m]/2pi), and
sin(w(q+k)) = sin(wq)cos(wk) + cos(wq)sin(wk) is separable -> scores become a
rank-1024 PE matmul in bf16 (f32 matmul runs as two slow LOW_HIGH passes; bf16
is a single full-rate pass).

Angle path, in turns (t = w x / 2pi), all matmuls bf16 with exact operands:
  t0_psum   = diag(w/2pi) @ (x_hi + x_lo)      (PE; host splits x = hi+lo bf16)
  cos bank += 0.25 (rank-1 ones pass)          (PE; shifts the rounding point)
  n         = (t0 + MAGIC) - MAGIC             (DVE, f32 magic round -> bf16 ints)
  red_psum += (-I) @ n                         (PE; red in [-0.5, 0.5] turns)
  feat      = Sin(2pi * red)  -> bf16          (ScalarE, PSUM -> SBUF, pair-merged)
K-side banks are swapped (cos first) so chunk products pair sin with cos.
Amplitudes: DVE bf16 tensor_scalar with per-partition table. Softmax without
max-subtraction; denominator via a ones-column in V; DVE reciprocal normalizes.

Sharding: B=8 -> one batch per core, no collectives.
"""

import math

import numpy as np
import ml_dtypes

import concourse.bass as bass
import concourse.mybir as mybir
from concourse.bass_utils import run_bass_kernel_spmd

F32 = mybir.dt.float32
BF16 = mybir.dt.bfloat16
AF = mybir.ActivationFunctionType
ALU = mybir.AluOpType

# base fit (amplitudes refit below against bf16-exact frequencies)
W0 = [0.273822509, 0.825679394, 1.38832881, 1.96485759,
      2.55624192, 3.16272728, 3.77941797, 4.47596827]

B, L, D, M = 8, 512, 64, 8
PI = math.pi
TWO_PI = 2.0 * math.pi
MAGIC = 12582912.0  # 1.5 * 2^23

PAIRS = [(0, 1), (2, 3), (4, 5), (6, 7)]


def _bf(x):
    return np.asarray(x).astype(ml_dtypes.bfloat16)


def _fit_consts():
    w2pi = _bf(np.array(W0, np.float32) / TWO_PI).astype(np.float64)
    w_eff = w2pi * TWO_PI
    S = 9.8
    sg = np.linspace(-S, S, 4001)
    wts = np.exp(-(sg**2) / 4) + 0.02
    A = np.sin(np.outer(sg, w_eff)) * np.sqrt(wts)[:, None]
    c, *_ = np.linalg.lstsq(A, np.tanh(sg) * np.sqrt(wts), rcond=None)
    return w2pi.astype(np.float32), c.astype(np.float32)


W2PI, C = _fit_consts()

_CACHE = {}


def _build():
    nc = bass.Bass()
    qhl_ext = nc.declare_dram_parameter("qhl", [128, L], BF16, isOutput=False)
    khl_ext = nc.declare_dram_parameter("khl", [128, L], BF16, isOutput=False)
    vh_ext = nc.declare_dram_parameter("vh", [L, 65], BF16, isOutput=False)
    dg_ext = nc.declare_dram_parameter("dg", [128, 6, 128], BF16, isOutput=False)
    amp_ext = nc.declare_dram_parameter("amp", [128, 4], F32, isOutput=False)
    out_ext = nc.declare_dram_parameter("out", [L, D], F32, isOutput=True)

    from contextlib import ExitStack

    with ExitStack() as ctx:
        e = ctx.enter_context
        QHL = e(nc.sbuf_tensor("QHL", [128, L], BF16))
        KHL = e(nc.sbuf_tensor("KHL", [128, L], BF16))
        DG = e(nc.sbuf_tensor([128, 6, 128], BF16))  # 0-3: diag(w/2pi); 4: -I; 5: 0.25-col
        ONES = e(nc.sbuf_tensor([1, L], BF16))
        AMP = e(nc.sbuf_tensor([128, 4], F32))
        VH = e(nc.sbuf_tensor([128, 4, 65], BF16))
        NS = e(nc.sbuf_tensor([128, 2, 4, L], BF16))
        NC_ = e(nc.sbuf_tensor("NCT", [128, 2, 4, L], BF16))
        FQRAW = e(nc.sbuf_tensor([128, 4, 2, L], BF16))
        FQS = e(nc.sbuf_tensor([128, 4, 2, L], BF16))
        FK = e(nc.sbuf_tensor([128, 4, 2, L], BF16))
        EXPT = e(nc.sbuf_tensor([128, 4, L], BF16))
        RCP = e(nc.sbuf_tensor([128, 4], F32))
        OUT = e(nc.sbuf_tensor([128, 4, D], F32))
        PSUMS = e(nc.psum_tensor([128, 4 * L], F32))
        PSUMR = e(nc.psum_tensor([128, 4, L], F32))
        s_in = e(nc.semaphore("s_in"))
        s_vh = e(nc.semaphore("s_vh"))
        s_t0 = e(nc.semaphore("s_t0"))
        s_n = e(nc.semaphore("s_n"))
        s_red = e(nc.semaphore("s_red"))
        s_act = e(nc.semaphore("s_act"))
        s_amp = e(nc.semaphore("s_amp"))
        s_scores = e(nc.semaphore("s_scores"))
        s_exp = e(nc.semaphore("s_exp"))
        s_av = e(nc.semaphore("s_av"))
        s_norm = e(nc.semaphore("s_norm"))
        block = e(nc.Block())

        XHL = [QHL, KHL]

        # units: g = 2*pair + side (Q first). Banks: rA=(2g)%4, rB=rA+1.
        # Q: A=sin, B=cos; K: A=cos, B=sin (so FK comes out [cos|sin]).
        def banks(g):
            rA = (2 * g) % 4
            return rA, rA + 1

        @block.sync
        def _(sync):
            sync.dma_start(out=QHL[:], in_=qhl_ext[:]).then_inc(s_in, 16)
            sync.dma_start(out=DG[:], in_=dg_ext[:]).then_inc(s_in, 16)
            sync.dma_start(out=AMP[:], in_=amp_ext[:]).then_inc(s_in, 16)
            sync.wait_ge(s_norm, 4)
            sync.dma_start(
                out=out_ext.rearrange("(g p) c -> p g c", p=128), in_=OUT[:]
            ).then_inc(s_in, 16)

        @block.gpsimd
        def _(gpsimd):
            gpsimd.memset(ONES[:], 1.0)
            gpsimd.sem_inc(s_in, 1)
            gpsimd.dma_start(out=KHL[:], in_=khl_ext[:]).then_inc(s_in, 16)
            gpsimd.dma_start(
                out=VH[:], in_=vh_ext.rearrange("(g p) c -> p g c", p=128)
            ).then_inc(s_vh, 16)

        @block.vector
        def _(vector):
            for g in range(8):
                j, s = g // 2, g % 2
                rA, rB = banks(g)
                sin_bank, cos_bank = (rA, rB) if s == 0 else (rB, rA)
                vector.wait_ge(s_t0, 2 * g + 1)
                vector.tensor_scalar(
                    NS[:, s, j, :], PSUMR[:, sin_bank, :], MAGIC, -MAGIC,
                    ALU.add, ALU.add,
                ).then_inc(s_n, 1)
                vector.wait_ge(s_t0, 2 * g + 2)
                vector.tensor_scalar(
                    NC_[:, s, j, :], PSUMR[:, cos_bank, :], MAGIC, -MAGIC,
                    ALU.add, ALU.add,
                ).then_inc(s_n, 1)
                if s == 1:
                    vector.wait_ge(s_act, 2 * j + 1)
                    vector.tensor_scalar_mul(
                        FQS[:, j], FQRAW[:, j], AMP[:, j : j + 1]
                    ).then_inc(s_amp, 1)
            for ib in range(4):
                vector.wait_ge(s_av, ib + 1)
                vector.reciprocal(RCP[:, ib : ib + 1], PSUMR[:, ib, 64:65])
            vector.drain()
            for ib in range(4):
                vector.tensor_scalar_mul(
                    OUT[:, ib, :], PSUMR[:, ib, 0:D], RCP[:, ib : ib + 1]
                ).then_inc(s_norm, 1)

        @block.scalar
        def _(scalar):
            for g in range(8):
                j, s = g // 2, g % 2
                rA, _ = banks(g)
                dst = FQRAW[:, j] if s == 0 else FK[:, j]
                scalar.wait_ge(s_red, 2 * g + 2)
                scalar.activation(
                    dst, PSUMR[:, rA : rA + 2, :], AF.Sin, scale=TWO_PI
                ).then_inc(s_act, 1)
            for jb in range(4):
                scalar.wait_ge(s_scores, jb + 1)
                scalar.activation(
                    EXPT[:, jb], PSUMS[:, jb * L : (jb + 1) * L], AF.Exp
                ).then_inc(s_exp, 1)

        @block.tensor
        def _(tensor):
            def red_unit(g):
                j, s = g // 2, g % 2
                rA, rB = banks(g)
                sin_bank, cos_bank = (rA, rB) if s == 0 else (rB, rA)
                tensor.wait_ge(s_in, 65)
                if g >= 2:
                    tensor.wait_ge(s_act, g - 1)  # banks free after act of g-2
                tensor.matmul(PSUMR[:, sin_bank, :], DG[:, j, :], XHL[s][:],
                              start=True, stop=False).then_inc(s_t0, 1)
                tensor.matmul(PSUMR[:, cos_bank, :], DG[:, j, :], XHL[s][:],
                              start=True, stop=False)
                tensor.matmul(PSUMR[:, cos_bank, :], DG[:1, 5, :], ONES[:],
                              start=False, stop=False).then_inc(s_t0, 1)
                tensor.wait_ge(s_n, 2 * g + 1)
                tensor.matmul(PSUMR[:, sin_bank, :], DG[:, 4, :], NS[:, s, j, :],
                              start=False, stop=True).then_inc(s_red, 1)
                tensor.wait_ge(s_n, 2 * g + 2)
                tensor.matmul(PSUMR[:, cos_bank, :], DG[:, 4, :], NC_[:, s, j, :],
                              start=False, stop=True).then_inc(s_red, 1)

            # HAM warmup: junk matmuls during the DMA window get the PE to 2.4GHz
            for _ in range(7):
                tensor.matmul(PSUMS[:, 0:L], DG[:, 0, :], QHL[:],
                              start=True, stop=True)

            def scores(j):
                tensor.wait_ge(s_amp, j + 1)
                tensor.wait_ge(s_act, 2 * j + 2)
                for t in range(2):
                    for jb in range(4):
                        mm = tensor.matmul(
                            PSUMS[:, jb * L : (jb + 1) * L],
                            FK[:, j, t, jb * 128 : (jb + 1) * 128],
                            FQS[:, j, t, :],
                            start=(j == 0 and t == 0),
                            stop=(j == 3 and t == 1),
                        )
                        if j == 3 and t == 1:
                            mm.then_inc(s_scores, 1)
                return mm

            for g in range(2):
                red_unit(g)
            for j in range(4):
                for g in (2 * j + 2, 2 * j + 3):
                    if g < 8:
                        red_unit(g)
                mm = scores(j)

            tensor.wait_ge(s_vh, 16)
            for jb in range(4):
                tensor.wait_ge(s_exp, jb + 1)
                for ib in range(4):
                    mm = tensor.matmul(
                        PSUMR[:, ib, 0:65],
                        EXPT[:, jb, ib * 128 : (ib + 1) * 128],
                        VH[:, jb, :],
                        start=(jb == 0),
                        stop=(jb == 3),
                    )
                    if jb == 3:
                        mm.then_inc(s_av, 1)

    return nc


def _get_nc():
    if "nc" not in _CACHE:
        _CACHE["nc"] = _build()
    return _CACHE["nc"]


def _make_consts():
    dg = np.zeros((128, 6, 128), np.float32)
    amp = np.zeros((128, 4), np.float32)
    for j, (a, b) in enumerate(PAIRS):
        for p in range(64):
            # out col p (freq a, d=p) reads xh row p and xl row 64+p
            dg[p, j, p] = W2PI[a]
            dg[64 + p, j, p] = W2PI[a]
            # out col 64+p (freq b, d=p) reads the same rows
            dg[p, j, 64 + p] = W2PI[b]
            dg[64 + p, j, 64 + p] = W2PI[b]
        amp[0:64, j] = C[a]
        amp[64:128, j] = C[b]
    for p in range(128):
        dg[p, 4, p] = -1.0
    dg[0, 5, :] = 0.25
    return _bf(dg), amp


def _make_in_maps(q, k, v):
    dg, amp = _make_consts()
    in_maps = []
    for b in range(B):
        def hilo(x):
            xt = np.ascontiguousarray(x.T.astype(np.float32))      # [64, 512]
            h = _bf(xt)
            lo = _bf(xt - h.astype(np.float32))
            return np.concatenate([h, lo], axis=0)                  # [128, 512]

        qhl = hilo(q[b])
        khl = hilo(k[b])
        vh = _bf(np.concatenate(
            [v[b].astype(np.float32), np.ones((L, 1), np.float32)], axis=1
        ))
        in_maps.append({"qhl": qhl, "khl": khl,
                        "vh": vh, "dg": dg, "amp": amp})
    return in_maps


def _run(in_maps, **kw):
    nc = _get_nc()
    return run_bass_kernel_spmd(nc, in_maps, core_ids=list(range(8)), **kw)


def kernel(q: np.ndarray, k: np.ndarray, v: np.ndarray) -> np.ndarray:
    res = _run(_make_in_maps(q, k, v))
    out = np.stack([res.results[b]["out"] for b in range(B)]).astype(np.float32)
    return out


# revision 14
# speedup vs baseline: 2.1511x; 1.2693x over previous
"""Additive attention kernel for 8 Trainium2 NeuronCores.

Math: scores[b,i,j] = sum_d tanh(q[b,i,d] + k[b,j,d]); out = softmax_j(scores) @ v.

tanh(s) ~= sum_m C[m] sin(W[m] s) (M=8, refit with bf16-exact W[m]/2pi), and
sin(w(q+k)) = sin(wq)cos(wk) + cos(wq)sin(wk) is separable -> scores become a
rank-1024 PE matmul in bf16 (f32 matmul runs as two slow LOW_HIGH passes; bf16
is a single full-rate pass).

Angle path, in turns (t = w x / 2pi), all matmuls bf16 with exact operands:
  t0_psum   = diag(w/2pi) @ (x_hi + x_lo)      (PE; host splits x = hi+lo bf16)
  cos bank += 0.25 (rank-1 ones pass)          (PE; shifts the rounding point)
  n         = (t0 + MAGIC) - MAGIC             (DVE, f32 magic round -> bf16 ints)
  red_psum += (-I) @ n                         (PE; red in [-0.5, 0.5] turns)
  feat      = Sin(2pi * red)  -> bf16          (ScalarE, PSUM -> SBUF, pair-merged)
K-side banks are swapped (cos first) so chunk products pair sin with cos.
Amplitudes: DVE bf16 tensor_scalar with per-partition table. Softmax without
max-subtraction; denominator via a ones-column in V; DVE reciprocal normalizes.

Sharding: B=8 -> one batch per core, no collectives.
"""

import math

import numpy as np
import ml_dtypes

import concourse.bass as bass
import concourse.mybir as mybir
from concourse.bass_utils import run_bass_kernel_spmd

F32 = mybir.dt.float32
BF16 = mybir.dt.bfloat16
AF = mybir.ActivationFunctionType
ALU = mybir.AluOpType

# base fit (amplitudes refit below against bf16-exact frequencies)
W0 = [0.273822509, 0.825679394, 1.38832881, 1.96485759,
      2.55624192, 3.16272728, 3.77941797, 4.47596827]

B, L, D, M = 8, 512, 64, 8
PI = math.pi
TWO_PI = 2.0 * math.pi
MAGIC = 12582912.0  # 1.5 * 2^23

PAIRS = [(0, 1), (2, 3), (4, 5), (6, 7)]


def _bf(x):
    return np.asarray(x).astype(ml_dtypes.bfloat16)


def _fit_consts():
    w2pi = _bf(np.array(W0, np.float32) / TWO_PI).astype(np.float64)
    w_eff = w2pi * TWO_PI
    S = 9.8
    sg = np.linspace(-S, S, 4001)
    wts = np.exp(-(sg**2) / 4) + 0.02
    A = np.sin(np.outer(sg, w_eff)) * np.sqrt(wts)[:, None]
    c, *_ = np.linalg.lstsq(A, np.tanh(sg) * np.sqrt(wts), rcond=None)
    return w2pi.astype(np.float32), c.astype(np.float32)


W2PI, C = _fit_consts()

_CACHE = {}


def _build():
    nc = bass.Bass()
    qhl_ext = nc.declare_dram_parameter("qhl", [128, L], BF16, isOutput=False)
    khl_ext = nc.declare_dram_parameter("khl", [128, L], BF16, isOutput=False)
    vh_ext = nc.declare_dram_parameter("vh", [L, 65], BF16, isOutput=False)
    dg_ext = nc.declare_dram_parameter("dg", [128, 6, 128], BF16, isOutput=False)
    amp_ext = nc.declare_dram_parameter("amp", [128, 4], F32, isOutput=False)
    out_ext = nc.declare_dram_parameter("out", [L, D], F32, isOutput=True)

    from contextlib import ExitStack

    with ExitStack() as ctx:
        e = ctx.enter_context
        QHL = e(nc.sbuf_tensor("QHL", [128, L], BF16))
        KHL = e(nc.sbuf_tensor("KHL", [128, L], BF16))
        DG = e(nc.sbuf_tensor([128, 6, 128], BF16))  # 0-3: diag(w/2pi); 4: -I; 5: 0.25-col
        ONES = e(nc.sbuf_tensor([1, L], BF16))
        AMP = e(nc.sbuf_tensor([128, 4], F32))
        VH = e(nc.sbuf_tensor([128, 4, 65], BF16))
        NS = e(nc.sbuf_tensor([128, 2, 4, L], BF16))
        NC_ = e(nc.sbuf_tensor("NCT", [128, 2, 4, L], BF16))
        AC = e(nc.sbuf_tensor([128, 2, 4, L], F32))
        FQRAW = e(nc.sbuf_tensor([128, 4, 2, L], BF16))
        FQS = e(nc.sbuf_tensor([128, 4, 2, L], BF16))
        FK = e(nc.sbuf_tensor([128, 4, 2, L], BF16))
        EXPT = e(nc.sbuf_tensor([128, 4, L], BF16))
        RCP = e(nc.sbuf_tensor([128, 4], F32))
        OUT = e(nc.sbuf_tensor([128, 4, D], F32))
        PSUMS = e(nc.psum_tensor([128, 4 * L], F32))
        PSUMR = e(nc.psum_tensor([128, 4, L], F32))
        s_in = e(nc.semaphore("s_in"))
        s_vh = e(nc.semaphore("s_vh"))
        s_t0 = e(nc.semaphore("s_t0"))
        s_n = e(nc.semaphore("s_n"))
        s_red = e(nc.semaphore("s_red"))
        s_act = e(nc.semaphore("s_act"))
        s_amp = e(nc.semaphore("s_amp"))
        s_scores = e(nc.semaphore("s_scores"))
        s_exp = e(nc.semaphore("s_exp"))
        s_av = e(nc.semaphore("s_av"))
        s_norm = e(nc.semaphore("s_norm"))
        block = e(nc.Block())

        XHL = [QHL, KHL]

        # units: g = 2*pair + side (Q first). Banks: rA=(2g)%4, rB=rA+1.
        # Q: A=sin, B=cos; K: A=cos, B=sin (so FK comes out [cos|sin]).
        def banks(g):
            rA = (2 * g) % 4
            return rA, rA + 1

        @block.sync
        def _(sync):
            sync.dma_start(out=QHL[:], in_=qhl_ext[:]).then_inc(s_in, 16)
            sync.dma_start(out=DG[:], in_=dg_ext[:]).then_inc(s_in, 16)
            sync.dma_start(out=AMP[:], in_=amp_ext[:]).then_inc(s_in, 16)
            sync.wait_ge(s_norm, 4)
            sync.dma_start(
                out=out_ext.rearrange("(g p) c -> p g c", p=128), in_=OUT[:]
            ).then_inc(s_in, 16)

        @block.gpsimd
        def _(gpsimd):
            gpsimd.memset(ONES[:], 1.0)
            gpsimd.sem_inc(s_in, 1)
            gpsimd.dma_start(out=KHL[:], in_=khl_ext[:]).then_inc(s_in, 16)
            gpsimd.dma_start(
                out=VH[:], in_=vh_ext.rearrange("(g p) c -> p g c", p=128)
            ).then_inc(s_vh, 16)

        @block.vector
        def _(vector):
            for g in range(8):
                j, s = g // 2, g % 2
                rA, rB = banks(g)
                sin_bank, cos_bank = (rA, rB) if s == 0 else (rB, rA)
                vector.wait_ge(s_t0, 2 * g + 1)
                vector.tensor_scalar(
                    NS[:, s, j, :], PSUMR[:, sin_bank, :], MAGIC, -MAGIC,
                    ALU.add, ALU.add,
                ).then_inc(s_n, 1)
                vector.wait_ge(s_t0, 2 * g + 2)
                vector.tensor_scalar(
                    AC[:, s, j, :], PSUMR[:, cos_bank, :], 0.25, MAGIC,
                    ALU.add, ALU.add,
                )
                vector.tensor_scalar(
                    NC_[:, s, j, :], AC[:, s, j, :], -MAGIC, -0.25,
                    ALU.add, ALU.add,
                ).then_inc(s_n, 1)
                if s == 1:
                    vector.wait_ge(s_act, 2 * j + 1)
                    vector.tensor_scalar_mul(
                        FQS[:, j], FQRAW[:, j], AMP[:, j : j + 1]
                    ).then_inc(s_amp, 1)
            for ib in range(4):
                vector.wait_ge(s_av, ib + 1)
                vector.reciprocal(RCP[:, ib : ib + 1], PSUMR[:, ib, 64:65])
            vector.drain()
            for ib in range(4):
                vector.tensor_scalar_mul(
                    OUT[:, ib, :], PSUMR[:, ib, 0:D], RCP[:, ib : ib + 1]
                ).then_inc(s_norm, 1)

        @block.scalar
        def _(scalar):
            for g in range(8):
                j, s = g // 2, g % 2
                rA, _ = banks(g)
                dst = FQRAW[:, j] if s == 0 else FK[:, j]
                scalar.wait_ge(s_red, 2 * g + 2)
                scalar.activation(
                    dst, PSUMR[:, rA : rA + 2, :], AF.Sin, scale=TWO_PI
                ).then_inc(s_act, 1)
            for jb in range(4):
                scalar.wait_ge(s_scores, jb + 1)
                scalar.activation(
                    EXPT[:, jb], PSUMS[:, jb * L : (jb + 1) * L], AF.Exp
                ).then_inc(s_exp, 1)

        @block.tensor
        def _(tensor):
            def red_t0(g):
                j, s = g // 2, g % 2
                rA, rB = banks(g)
                sin_bank, cos_bank = (rA, rB) if s == 0 else (rB, rA)
                tensor.wait_ge(s_in, 65)
                if g >= 2:
                    tensor.wait_ge(s_act, g - 1)  # banks free after act of g-2
                tensor.matmul(PSUMR[:, sin_bank, :], DG[:, j, :], XHL[s][:],
                              start=True, stop=False).then_inc(s_t0, 1)
                tensor.matmul(PSUMR[:, cos_bank, :], DG[:, j, :], XHL[s][:],
                              start=True, stop=False).then_inc(s_t0, 1)

            def red_fin(g):
                j, s = g // 2, g % 2
                rA, rB = banks(g)
                sin_bank, cos_bank = (rA, rB) if s == 0 else (rB, rA)
                tensor.wait_ge(s_n, 2 * g + 1)
                tensor.matmul(PSUMR[:, sin_bank, :], DG[:, 4, :], NS[:, s, j, :],
                              start=False, stop=True).then_inc(s_red, 1)
                tensor.wait_ge(s_n, 2 * g + 2)
                tensor.matmul(PSUMR[:, cos_bank, :], DG[:, 4, :], NC_[:, s, j, :],
                              start=False, stop=True).then_inc(s_red, 1)

            def scores(j):
                tensor.wait_ge(s_amp, j + 1)
                tensor.wait_ge(s_act, 2 * j + 2)
                for t in range(2):
                    for jb in range(4):
                        mm = tensor.matmul(
                            PSUMS[:, jb * L : (jb + 1) * L],
                            FK[:, j, t, jb * 128 : (jb + 1) * 128],
                            FQS[:, j, t, :],
                            start=(j == 0 and t == 0),
                            stop=(j == 3 and t == 1),
                        )
                        if j == 3 and t == 1:
                            mm.then_inc(s_scores, 1)
                return mm

            # two-deep pipeline: t0(g), t0(g+1), fin(g), fin(g+1), scores...
            red_t0(0)
            red_t0(1)
            red_fin(0)
            red_fin(1)
            for j in range(4):
                for g in (2 * j + 2, 2 * j + 3):
                    if g < 8:
                        red_t0(g)
                for g in (2 * j + 2, 2 * j + 3):
                    if g < 8:
                        red_fin(g)
                mm = scores(j)

            tensor.wait_ge(s_vh, 16)
            for jb in range(4):
                tensor.wait_ge(s_exp, jb + 1)
                for ib in range(4):
                    mm = tensor.matmul(
                        PSUMR[:, ib, 0:65],
                        EXPT[:, jb, ib * 128 : (ib + 1) * 128],
                        VH[:, jb, :],
                        start=(jb == 0),
                        stop=(jb == 3),
                    )
                    if jb == 3:
                        mm.then_inc(s_av, 1)

    return nc


def _get_nc():
    if "nc" not in _CACHE:
        _CACHE["nc"] = _build()
    return _CACHE["nc"]


def _make_consts():
    dg = np.zeros((128, 6, 128), np.float32)
    amp = np.zeros((128, 4), np.float32)
    for j, (a, b) in enumerate(PAIRS):
        for p in range(64):
            # out col p (freq a, d=p) reads xh row p and xl row 64+p
            dg[p, j, p] = W2PI[a]
            dg[64 + p, j, p] = W2PI[a]
            # out col 64+p (freq b, d=p) reads the same rows
            dg[p, j, 64 + p] = W2PI[b]
            dg[64 + p, j, 64 + p] = W2PI[b]
        amp[0:64, j] = C[a]
        amp[64:128, j] = C[b]
    for p in range(128):
        dg[p, 4, p] = -1.0
    dg[0, 5, :] = 0.25
    return _bf(dg), amp


def _make_in_maps(q, k, v):
    dg, amp = _make_consts()
    in_maps = []
    for b in range(B):
        def hilo(x):
            xt = np.ascontiguousarray(x.T.astype(np.float32))      # [64, 512]
            h = _bf(xt)
            lo = _bf(xt - h.astype(np.float32))
            return np.concatenate([h, lo], axis=0)                  # [128, 512]

        qhl = hilo(q[b])
        khl = hilo(k[b])
        vh = _bf(np.concatenate(
            [v[b].astype(np.float32), np.ones((L, 1), np.float32)], axis=1
        ))
        in_maps.append({"qhl": qhl, "khl": khl,
                        "vh": vh, "dg": dg, "amp": amp})
    return in_maps


def _run(in_maps, **kw):
    nc = _get_nc()
    return run_bass_kernel_spmd(nc, in_maps, core_ids=list(range(8)), **kw)


def kernel(q: np.ndarray, k: np.ndarray, v: np.ndarray) -> np.ndarray:
    res = _run(_make_in_maps(q, k, v))
    out = np.stack([res.results[b]["out"] for b in range(B)]).astype(np.float32)
    return out
